# revision 1
# baseline (speedup 1.0000x reference)
"""VQ codebook lookup (BagOfConcepts) on 8 TRN2 NeuronCores.

Data-parallel: shard flat tokens N=32768 across 8 cores (4096 each),
replicate the (4096, 512) codebook.

Default mode "fp8ship":
  Device computes the full (4096 x 4096) score matrix per core with
  fp8e4m3 DoubleRow matmuls (2 contraction chunks packed per instruction,
  256 rows/instr at 0.5 cyc/row), evicts PSUM to fp16 via ACT/DVE split,
  and streams the fp16 scores to DRAM.  Host then takes the top-16
  candidates per token (approx scores are ~3e-4 accurate vs a top-2 gap
  of ~1e-3, so the true argmin is always contained - 0 misses on the
  actual dataset even at top-8) and rescores ONLY those 16 in exact
  reference arithmetic (fp32-rounded d2 with first-index tie-break),
  then gathers codebook rows.

Fallback mode "fp16dr" (bit-exact on device, slower): fp16 main matmul
+ fp8 DoubleRow hi/lo corrections; argmax via DVE max/max_index on the
reference's fp32 rounding grid; SWDGE gather.
"""
import os
import numpy as np

B = 8
T = 4096
D = 512
K = 4096
NCORES = 8
P = 128
N = (B * T) // NCORES        # tokens per core = 4096
NT = N // P                  # 32 token tiles
NCH = D // P                 # 4 contraction chunks
NCP = NCH // 2               # 2 chunk-pairs (DoubleRow packs 2 chunks)
NKT = K // 512               # 8 k-tiles of 512
CSCALE = float(2.0 ** 12)    # codebook prescale for fp16 splits (fp16dr)

XS = 2.0 ** 4                # fp8 x prescale   (|x|<6 -> <96, e4m3 max 240)
CS = 2.0 ** 16               # fp8 c prescale   (|c|<2.5e-4 -> <16)
SCORE_SCALE = 2.0 ** -11     # evict scale: psum = mm*2^20 -> ship mm*2^9 fp8
SCORE_DESCALE = np.float32(2.0 ** -9)
MARGIN = np.float32(2.5e-3)  # host candidate margin below per-row approx max

MODE = os.environ.get("VQ_MODE", "fp8ship")

_CACHE = {}
LAST_RESULT = None


def _build_fp8ship():
    import concourse.bass as bass
    import concourse.mybir as mybir
    from concourse import bacc
    from concourse.tile import TileContext

    dt = mybir.dt
    f16 = dt.float16
    f8 = dt.float8e4

    nc = bacc.Bacc("TRN2", target_bir_lowering=False, debug=False)

    d_x8 = nc.dram_tensor("x8", [P, NT, NCP, 2, P], f8, kind="ExternalInput").ap()
    d_c8 = nc.dram_tensor("c8", [P, NKT, NCP, 2, 512], f8, kind="ExternalInput").ap()
    d_sc = nc.dram_tensor("sc", [N, K], f8, kind="ExternalOutput").ap()

    LOOKAHEAD = 3
    with TileContext(nc) as tc:
        with (
            tc.tile_pool(name="const", bufs=1) as cpool,
            tc.tile_pool(name="xt", bufs=LOOKAHEAD + 2) as xtp,
            tc.tile_pool(name="score", bufs=4) as scp,
            tc.tile_pool(name="psum", bufs=4, space="PSUM") as psp,
        ):
            x8_tiles = {}

            def fetch_x8(i):
                if i < NT:
                    t = xtp.tile([P, NCP, 2, P], f8, tag="x8")
                    nc.scalar.dma_start(t[:], d_x8[:, i, :, :, :])
                    x8_tiles[i] = t

            fetch_x8(0)
            c8_tiles = []
            for kt in range(NKT):
                t = cpool.tile([P, NCP, 2, 512], f8, tag=f"c8{kt}", name=f"c8{kt}")
                nc.scalar.dma_start(t[:], d_c8[:, kt, :, :, :])
                c8_tiles.append(t)
                if kt < LOOKAHEAD:
                    fetch_x8(kt + 1)

            for i in range(NT):
                fetch_x8(i + LOOKAHEAD + 1)
                x8_t = x8_tiles.pop(i)
                score_t = scp.tile([P, K], f8, tag="score")
                for q in range(4):
                    ph = psp.tile([P, 1024], dt.float32, tag="ph")
                    for cp in range(NCP):
                        for s in range(2):
                            kt = q * 2 + s
                            nc.tensor.matmul(
                                ph[:, s * 512:(s + 1) * 512],
                                lhsT=x8_t[:, cp, :, :],
                                rhs=c8_tiles[kt][:, cp, :, :],
                                start=(cp == 0), stop=(cp == NCP - 1),
                                perf_mode=mybir.MatmulPerfMode.DoubleRow,
                            )
                    lo, hi = q * 1024, (q + 1) * 1024
                    if q % 2 == 0:
                        nc.scalar.activation(
                            score_t[:, lo:hi], ph[:],
                            mybir.ActivationFunctionType.Identity,
                            scale=SCORE_SCALE,
                        )
                    else:
                        nc.vector.tensor_scalar_mul(
                            score_t[:, lo:hi], ph[:], SCORE_SCALE,
                        )
                    if q == 1:
                        nc.sync.dma_start(
                            d_sc[i * P:(i + 1) * P, 0:2048], score_t[:, 0:2048])
                    elif q == 3:
                        nc.sync.dma_start(
                            d_sc[i * P:(i + 1) * P, 2048:4096], score_t[:, 2048:4096])

    nc.compile()
    return nc


def _build_fp16dr():
    import concourse.bass as bass
    import concourse.mybir as mybir
    from concourse import bacc
    from concourse.tile import TileContext

    dt = mybir.dt
    f32 = dt.float32
    f16 = dt.float16
    f8 = dt.float8e4

    nc = bacc.Bacc("TRN2", target_bir_lowering=False, debug=False)

    d_xm = nc.dram_tensor("xm", [P, NT, NCH, P], f16, kind="ExternalInput").ap()
    d_x8 = nc.dram_tensor("x8", [P, NT, NCH, 2, P], f8, kind="ExternalInput").ap()
    d_cm = nc.dram_tensor("cm", [P, NKT, NCH, 512], f16, kind="ExternalInput").ap()
    d_c8 = nc.dram_tensor("c8", [P, NKT, NCH, 2, 512], f8, kind="ExternalInput").ap()
    d_xn = nc.dram_tensor("xn", [N, D], f32, kind="ExternalInput").ap()
    d_cn = nc.dram_tensor("cn", [K, D], f32, kind="ExternalInput").ap()
    d_out = nc.dram_tensor("out", [N, D], f32, kind="ExternalOutput").ap()

    step1_scale = 2.0 ** -32  # PSUM holds mm * 2^33

    with TileContext(nc) as tc:
        with (
            tc.tile_pool(name="const", bufs=1) as cpool,
            tc.tile_pool(name="xt", bufs=4) as xtp,
            tc.tile_pool(name="xn", bufs=3) as xnp_,
            tc.tile_pool(name="sq", bufs=2) as sqp,
            tc.tile_pool(name="rs", bufs=4) as rsp,
            tc.tile_pool(name="score", bufs=3) as scp,
            tc.tile_pool(name="top", bufs=3) as topp,
            tc.tile_pool(name="gat", bufs=4) as gatp,
            tc.tile_pool(name="psum", bufs=2, space="PSUM") as psp,
        ):
            cm_tiles, c8_tiles = [], []
            for kt in range(NKT):
                tm = cpool.tile([P, NCH, 512], f16, tag=f"cm{kt}", name=f"cm{kt}")
                nc.scalar.dma_start(tm[:], d_cm[:, kt, :, :])
                cm_tiles.append(tm)
                t8 = cpool.tile([P, NCH, 2, 512], f8, tag=f"c8{kt}", name=f"c8{kt}")
                nc.scalar.dma_start(t8[:], d_c8[:, kt, :, :, :])
                c8_tiles.append(t8)

            for i in range(NT):
                xm_t = xtp.tile([P, NCH, P], f16, tag="xm")
                nc.sync.dma_start(xm_t[:], d_xm[:, i, :, :])
                x8_t = xtp.tile([P, NCH, 2, P], f8, tag="x8")
                nc.sync.dma_start(x8_t[:], d_x8[:, i, :, :, :])

                xn_t = xnp_.tile([P, D], f32, tag="xn")
                nc.sync.dma_start(xn_t[:], d_xn[i * P:(i + 1) * P, :])
                sq_t = sqp.tile([P, D], f32, tag="sq")
                rs_t = rsp.tile([P, 1], f32, tag="rs")
                nc.scalar.activation(
                    sq_t[:], xn_t[:], mybir.ActivationFunctionType.Square,
                    accum_out=rs_t[:],
                )
                rsn_t = rsp.tile([P, 1], f32, tag="rsn")
                nc.gpsimd.tensor_scalar_mul(rsn_t[:], rs_t[:], -1.0)

                score_t = scp.tile([P, K], f32, tag="score")
                HKQ = 2048
                for h in range(K // HKQ):
                    ph = psp.tile([P, HKQ], f32, tag="ph")
                    for c in range(NCH):
                        for s in range(HKQ // 512):
                            kt = (h * HKQ) // 512 + s
                            nc.tensor.matmul(
                                ph[:, s * 512:(s + 1) * 512],
                                lhsT=xm_t[:, c, :],
                                rhs=cm_tiles[kt][:, c, :],
                                start=(c == 0), stop=False,
                            )
                    for c in range(NCH):
                        for s in range(HKQ // 512):
                            kt = (h * HKQ) // 512 + s
                            nc.tensor.matmul(
                                ph[:, s * 512:(s + 1) * 512],
                                lhsT=x8_t[:, c, :, :],
                                rhs=c8_tiles[kt][:, c, :, :],
                                start=False, stop=(c == NCH - 1),
                                perf_mode=mybir.MatmulPerfMode.DoubleRow,
                            )
                    nc.scalar.activation(
                        score_t[:, h * HKQ:(h + 1) * HKQ], ph[:],
                        mybir.ActivationFunctionType.Identity,
                        bias=rsn_t[:, 0:1], scale=step1_scale,
                    )

                max8 = topp.tile([P, 8], f32, tag="max8")
                idx8 = topp.tile([P, 8], dt.uint32, tag="idx8")
                nc.vector.max(out=max8[:], in_=score_t[:])
                nc.vector.max_index(out=idx8[:], in_max=max8[:], in_values=score_t[:])

                gat_t = gatp.tile([P, D], f32, tag="gat")
                nc.gpsimd.indirect_dma_start(
                    out=gat_t[:], out_offset=None, in_=d_cn[:],
                    in_offset=bass.IndirectOffsetOnAxis(ap=idx8[:, 0:1], axis=0),
                )
                nc.sync.dma_start(d_out[i * P:(i + 1) * P, :], gat_t[:])

    nc.compile()
    return nc


def _get_nc(mode):
    if mode not in _CACHE:
        _CACHE[mode] = _build_fp8ship() if mode == "fp8ship" else _build_fp16dr()
    return _CACHE[mode]


def _prep_xt(x):
    # x: [N, D] -> [P, NT, NCH, P] (partition=d%128, token-tile, d-chunk, token)
    return np.ascontiguousarray(
        x.T.reshape(NCH, P, NT, P).transpose(1, 2, 0, 3)
    )


def _prep_ct(c):
    # c: [K, D] -> [P, NKT, NCH, 512]
    return np.ascontiguousarray(
        c.T.reshape(NCH, P, NKT, 512).transpose(1, 2, 0, 3)
    )


def _prep_x8_pairs(x8):
    # x8: [N, D] fp8 -> [P, NT, NCP, 2, P]; d = (2*cp+q)*128 + p, token = 128*i+t
    a = x8.T.reshape(NCP, 2, P, NT, P)          # [cp, q, p, i, t]
    return np.ascontiguousarray(a.transpose(2, 3, 0, 1, 4))


def _prep_c8_pairs(c8):
    # c8: [K, D] fp8 -> [P, NKT, NCP, 2, 512]
    a = c8.T.reshape(NCP, 2, P, NKT, 512)       # [cp, q, p, kt, kcol]
    return np.ascontiguousarray(a.transpose(2, 3, 0, 1, 4))


def _run_spmd(nc, in_maps):
    from concourse.bass_utils import run_bass_kernel_spmd
    try:
        return run_bass_kernel_spmd(nc, in_maps, core_ids=list(range(NCORES)))
    except ModuleNotFoundError:
        # tracing requested but axon ntff hook unavailable in this container
        os.environ["BASS_NEVER_TRACE"] = "1"
        return run_bass_kernel_spmd(nc, in_maps, core_ids=list(range(NCORES)))


def _kernel_fp8ship(inp, codebook):
    global LAST_RESULT
    import ml_dtypes
    f8np = ml_dtypes.float8_e4m3
    f32 = np.float32

    flat = inp.reshape(-1, D)                      # [32768, 512]
    shards = flat.reshape(NCORES, N, D)

    nc = _get_nc("fp8ship")

    c8 = (codebook * f32(CS)).astype(f8np)
    c8_p = _prep_c8_pairs(c8).view(np.uint8)
    in_maps = []
    for s in range(NCORES):
        x8 = (shards[s] * f32(XS)).astype(f8np)
        in_maps.append({"x8": _prep_x8_pairs(x8).view(np.uint8), "c8": c8_p})

    res = _run_spmd(nc, in_maps)
    LAST_RESULT = res
    sc_raw = [np.asarray(r["sc"]).view(np.uint8) for r in res.results]
    scores_u8 = np.concatenate(sc_raw, axis=0)        # [32768, 4096] e4m3 bytes

    # host: candidates = approx scores within MARGIN of each row max
    # (fp8 matmul+quant noise sigma ~4.6e-4 pairwise; margin = 5.4 sigma),
    # then exact rescore in reference arithmetic with first-index tie-break.
    NTOK = flat.shape[0]
    lut = (np.arange(256, dtype=np.uint8).view(ml_dtypes.float8_e4m3)
           .astype(f32) * SCORE_DESCALE)              # e4m3 byte -> score
    s = lut[scores_u8]                                # [32768, 4096] f32
    rowmax = s.max(axis=1)
    rows, cols = np.nonzero(s >= (rowmax[:, None] - MARGIN))

    x64 = flat.astype(np.float64)
    s1 = np.einsum("nd,nd->n", x64, x64).astype(f32)
    mm = np.einsum("id,id->i", flat[rows], codebook[cols])
    d2 = s1[rows] - f32(2.0) * mm
    order = np.lexsort((cols, d2, rows))
    rs = rows[order]
    first = np.searchsorted(rs, np.arange(NTOK))
    win = cols[order][first]

    return codebook[win].reshape(inp.shape).astype(np.float32)


def _kernel_fp16dr(inp, codebook):
    global LAST_RESULT
    import ml_dtypes
    f8np = ml_dtypes.float8_e4m3
    f32 = np.float32

    flat = inp.reshape(-1, D)
    shards = flat.reshape(NCORES, N, D)
    nc = _get_nc("fp16dr")

    cs = codebook * f32(CSCALE)              # c * 2^12
    ch = cs.astype(np.float16)
    cl = (cs - ch.astype(f32)).astype(np.float16)
    cm = (ch.astype(f32) * f32(2.0 ** 10)).astype(np.float16)   # exact
    cl8 = (cl.astype(f32) * f32(2.0 ** 17)).astype(f8np)
    ch8 = (ch.astype(f32) * f32(2.0 ** 6)).astype(f8np)
    cm_p = _prep_ct(cm)
    c8_p = np.stack([_prep_ct(cl8), _prep_ct(ch8)], axis=3).view(np.uint8)
    in_maps = []
    for s in range(NCORES):
        x = shards[s]
        xh = x.astype(np.float16)
        xl = (x - xh.astype(f32)).astype(np.float16)
        xm = (xh.astype(f32) * f32(2.0 ** 11)).astype(np.float16)  # exact
        xh8 = (xh.astype(f32) * f32(2.0 ** 4)).astype(f8np)
        xl8 = (xl.astype(f32) * f32(2.0 ** 15)).astype(f8np)
        x8_p = np.stack([_prep_xt(xh8), _prep_xt(xl8)], axis=3).view(np.uint8)
        in_maps.append({
            "xm": _prep_xt(xm), "x8": x8_p,
            "xn": np.ascontiguousarray(x),
            "cm": cm_p, "c8": c8_p, "cn": codebook,
        })

    res = _run_spmd(nc, in_maps)
    LAST_RESULT = res
    out = np.stack([r["out"] for r in res.results])   # [8, 4096, 512]
    return out.reshape(inp.shape).astype(np.float32)


def kernel(inp, codebook):
    inp = np.asarray(inp, dtype=np.float32)
    codebook = np.asarray(codebook, dtype=np.float32)
    if MODE == "fp8ship":
        return _kernel_fp8ship(inp, codebook)
    return _kernel_fp16dr(inp, codebook)



# revision 54
# speedup vs baseline: 1.0704x; 1.0704x over previous
"""VQ codebook lookup (BagOfConcepts) on 8 TRN2 NeuronCores.

Data-parallel: shard flat tokens N=32768 across 8 cores (4096 each),
replicate the (4096, 512) codebook.

Default mode "fp8ship":
  Device computes the full (4096 x 4096) score matrix per core with
  fp8e4m3 DoubleRow matmuls (2 contraction chunks packed per instruction,
  256 rows/instr at 0.5 cyc/row), evicts PSUM to fp16 via ACT/DVE split,
  and streams the fp16 scores to DRAM.  Host then takes the top-16
  candidates per token (approx scores are ~3e-4 accurate vs a top-2 gap
  of ~1e-3, so the true argmin is always contained - 0 misses on the
  actual dataset even at top-8) and rescores ONLY those 16 in exact
  reference arithmetic (fp32-rounded d2 with first-index tie-break),
  then gathers codebook rows.

Fallback mode "fp16dr" (bit-exact on device, slower): fp16 main matmul
+ fp8 DoubleRow hi/lo corrections; argmax via DVE max/max_index on the
reference's fp32 rounding grid; SWDGE gather.
"""
import os
import numpy as np

B = 8
T = 4096
D = 512
K = 4096
NCORES = 8
P = 128
N = (B * T) // NCORES        # tokens per core = 4096
NT = N // P                  # 32 token tiles
NCH = D // P                 # 4 contraction chunks
NCP = NCH // 2               # 2 chunk-pairs (DoubleRow packs 2 chunks)
NKT = K // 512               # 8 k-tiles of 512
CSCALE = float(2.0 ** 12)    # codebook prescale for fp16 splits (fp16dr)

XS = 2.0 ** 4                # fp8 x prescale   (|x|<6 -> <96, e4m3 max 240)
CS = 2.0 ** 16               # fp8 c prescale   (|c|<2.5e-4 -> <16)
SCORE_SCALE = 2.0 ** -11     # evict scale: psum = mm*2^20 -> ship mm*2^9 fp8
SCORE_DESCALE = np.float32(2.0 ** -9)
MARGIN = np.float32(2.5e-3)  # host candidate margin below per-row approx max

MODE = os.environ.get("VQ_MODE", "gmax4")

_CACHE = {}
LAST_RESULT = None


def _build_fp8ship_v2():
    """Rebalanced fp8 score-ship kernel.

    vs the v1 baseline (92.2us TimelineSim):
      - PSUM evictions split across ALL THREE elementwise engines
        (ACT 800 / DVE 704 / Pool 544 cols per 2048-col half, ~853ns each,
        matching the PE's 853ns per half-tile) instead of ACT/DVE only.
      - one score DMA per token tile ([128,4096] fp8, 128x4096B
        descriptors) instead of two -> halves HWDGE fixed cost.
      - x8 fetched two tiles per DMA.
      - full 8-bank PSUM ping-pong (two 4-bank halves in flight).
    """
    import concourse.bass as bass
    import concourse.mybir as mybir
    from concourse import bacc
    from concourse.tile import TileContext

    dt = mybir.dt
    f8 = dt.float8e4

    nc = bacc.Bacc("TRN2", target_bir_lowering=False, debug=False)

    d_x8 = nc.dram_tensor("x8", [P, NT // 2, 2, NCP, 2, P], f8, kind="ExternalInput").ap()
    d_c8 = nc.dram_tensor("c8", [P, NKT, NCP, 2, 512], f8, kind="ExternalInput").ap()
    d_sc = nc.dram_tensor("sc", [N, K], f8, kind="ExternalOutput").ap()

    # Per 1024-col quarter (2 PSUM banks), ONE evictor engine. A single
    # reader per PSUM tile avoids tile's reader-chaining serialization.
    # Rotation weights ~ inverse eviction cost:
    #   ACT 1038ns / DVE 1192ns / Pool 1517ns per [128,1024] quarter.
    QSHARE = {"A": 0.385, "V": 0.335, "P": 0.28}

    def quarter_schedule(nq):
        # Bresenham-style weighted rotation
        acc = {k: 0.0 for k in QSHARE}
        seq = []
        for _ in range(nq):
            for k in QSHARE:
                acc[k] += QSHARE[k]
            pick = max(acc, key=lambda k: acc[k])
            acc[pick] -= 1.0
            seq.append(pick)
        return seq

    QSCHED = quarter_schedule(NT * 4)

    LOOKAHEAD = 2  # x8 tile-pair prefetch depth
    with TileContext(nc) as tc:
        with (
            tc.tile_pool(name="const", bufs=1) as cpool,
            tc.tile_pool(name="xt", bufs=LOOKAHEAD + 2) as xtp,
            tc.tile_pool(name="score", bufs=3) as scp,
            tc.tile_pool(name="psum", bufs=4, space="PSUM") as psp,
        ):
            x8_tiles = {}

            def fetch_x8(ip):  # fetch tile-pair ip (tiles 2*ip, 2*ip+1)
                if ip < NT // 2:
                    t = xtp.tile([P, 2, NCP, 2, P], f8, tag="x8")
                    nc.scalar.dma_start(t[:], d_x8[:, ip, :, :, :, :])
                    x8_tiles[ip] = t

            fetch_x8(0)
            c8_tiles = []
            for kt in range(NKT):
                t = cpool.tile([P, NCP, 2, 512], f8, tag=f"c8{kt}", name=f"c8{kt}")
                nc.scalar.dma_start(t[:], d_c8[:, kt, :, :, :])
                c8_tiles.append(t)
                if kt < LOOKAHEAD:
                    fetch_x8(kt + 1)

            for i in range(NT):
                if i % 2 == 0:
                    fetch_x8(i // 2 + LOOKAHEAD + 1)
                x8_pair = x8_tiles[i // 2]
                x8_t = x8_pair[:, i % 2]
                if i % 2 == 1:
                    x8_tiles.pop(i // 2)
                score_t = scp.tile([P, K], f8, tag="score")
                for q in range(4):
                    ph = psp.tile([P, 1024], dt.float32, tag="ph")
                    for cp in range(NCP):
                        for s in range(2):
                            kt = q * 2 + s
                            nc.tensor.matmul(
                                ph[:, s * 512:(s + 1) * 512],
                                lhsT=x8_t[:, cp, :, :],
                                rhs=c8_tiles[kt][:, cp, :, :],
                                start=(cp == 0), stop=(cp == NCP - 1),
                                perf_mode=mybir.MatmulPerfMode.DoubleRow,
                            )
                    lo = q * 1024
                    eng = QSCHED[i * 4 + q]
                    if eng == "A":
                        nc.scalar.activation(
                            score_t[:, lo:lo + 1024], ph[:],
                            mybir.ActivationFunctionType.Identity,
                            scale=SCORE_SCALE,
                        )
                    elif eng == "V":
                        nc.vector.tensor_scalar_mul(
                            score_t[:, lo:lo + 1024], ph[:], SCORE_SCALE,
                        )
                    else:
                        nc.gpsimd.tensor_scalar_mul(
                            score_t[:, lo:lo + 1024], ph[:], SCORE_SCALE,
                        )
                nc.sync.dma_start(d_sc[i * P:(i + 1) * P, :], score_t[:])

    nc.compile()
    return nc


def _build_fp8ship_v3():
    """1024-col DR matmuls + paired-engine quarter evictions.

    PE.SEQ relief: 8 matmuls + 4 ldweights per tile (vs 16+16), keeping
    the sequencer (~120ns/MM + 38ns/LDW) well under the 1707ns/tile of
    PE.ENGINE work.  Per 1024-col quarter one evictor engine; the two
    quarters of a half go to two DIFFERENT engines (pair rotation
    AV/AP/VP at 28/21/15 per 64) so they drain in parallel.
    """
    import concourse.bass as bass
    import concourse.mybir as mybir
    from concourse import bacc
    from concourse.tile import TileContext

    dt = mybir.dt
    f8 = dt.float8e4

    nc = bacc.Bacc("TRN2", target_bir_lowering=False, debug=False)

    d_x8 = nc.dram_tensor("x8", [P, NT // 2, 2, NCP, 2, P], f8, kind="ExternalInput").ap()
    # codebook as 4 super-ktiles of 1024 cols, DR-packed
    d_c8 = nc.dram_tensor("c8", [P, 4, NCP, 2, 1024], f8, kind="ExternalInput").ap()
    d_sc = nc.dram_tensor("sc", [N, K], f8, kind="ExternalOutput").ap()

    def pair_schedule(nh):
        w = {("A", "V"): 28 / 64, ("A", "P"): 21 / 64, ("V", "P"): 15 / 64}
        acc = {k: 0.0 for k in w}
        seq = []
        for _ in range(nh):
            for k in w:
                acc[k] += w[k]
            pick = max(acc, key=lambda k: acc[k])
            acc[pick] -= 1.0
            seq.append(pick)
        return seq

    HSCHED = pair_schedule(NT * 2)

    LOOKAHEAD = 2
    with TileContext(nc) as tc:
        with (
            tc.tile_pool(name="const", bufs=1) as cpool,
            tc.tile_pool(name="xt", bufs=LOOKAHEAD + 2) as xtp,
            tc.tile_pool(name="score", bufs=3) as scp,
            tc.tile_pool(name="psum", bufs=2, space="PSUM") as psp,
        ):
            x8_tiles = {}

            def fetch_x8(ip):
                if ip < NT // 2:
                    t = xtp.tile([P, 2, NCP, 2, P], f8, tag="x8")
                    nc.sync.dma_start(t[:], d_x8[:, ip, :, :, :, :])
                    x8_tiles[ip] = t

            fetch_x8(0)
            c8_tiles = []
            for kt in range(4):
                t = cpool.tile([P, NCP, 2, 1024], f8, tag=f"c8{kt}", name=f"c8{kt}")
                nc.sync.dma_start(t[:], d_c8[:, kt, :, :, :])
                c8_tiles.append(t)
                if kt < LOOKAHEAD:
                    fetch_x8(kt + 1)

            def evict(eng, dst, src):
                if eng == "A":
                    nc.scalar.activation(
                        dst, src, mybir.ActivationFunctionType.Identity,
                        scale=SCORE_SCALE)
                elif eng == "V":
                    nc.vector.tensor_scalar_mul(dst, src, SCORE_SCALE)
                else:
                    nc.gpsimd.tensor_scalar_mul(dst, src, SCORE_SCALE)

            for i in range(NT):
                if i % 2 == 0:
                    fetch_x8(i // 2 + LOOKAHEAD + 1)
                x8_pair = x8_tiles[i // 2]
                x8_t = x8_pair[:, i % 2]
                if i % 2 == 1:
                    x8_tiles.pop(i // 2)
                score_t = scp.tile([P, K], f8, tag="score")
                for h in range(2):
                    pha = psp.tile([P, 1024], dt.float32, tag="pha", name="pha")
                    phb = psp.tile([P, 1024], dt.float32, tag="phb", name="phb")
                    for cp in range(NCP):
                        for qq, ph in ((0, pha), (1, phb)):
                            nc.tensor.matmul(
                                ph[:],
                                lhsT=x8_t[:, cp, :, :],
                                rhs=c8_tiles[h * 2 + qq][:, cp, :, :],
                                start=(cp == 0), stop=(cp == NCP - 1),
                                perf_mode=mybir.MatmulPerfMode.DoubleRow,
                            )
                    e1, e2 = HSCHED[i * 2 + h]
                    lo = h * 2048
                    evict(e1, score_t[:, lo:lo + 1024], pha[:])
                    evict(e2, score_t[:, lo + 1024:lo + 2048], phb[:])
                nc.sync.dma_start(d_sc[i * P:(i + 1) * P, :], score_t[:])

    nc.compile()
    return nc


def _build_fp8ship_v4():
    """HW-legal rebalanced fp8 score ship.

    Constraints discovered on real TRN2 (walrus verifier):
      - TensorTensor/etc may read only ONE input from PSUM -> each PSUM
        tile gets exactly one consumer instruction (also avoids tile's
        reader-chain serialization).
    Per 128-token tile, 8 PSUM banks split [e0|e1|q23|q45|e6|e7] with
    positioned consumers chosen so each bank is drained before the next
    tile's matmul needs it (deadline = T + 213ns * position):
      pos0  e0  [512]  Pool evict  806ns
      pos1  e1  [512]  DVE  evict  658ns
      pos2-3 q23[1024] ACT  evict 1038ns
      pos4-5 q45[1024] DVE  evict 1192ns
      pos6  e6  [512]  Pool evict  806ns
      pos7  e7  [512]  ACT  evict  612ns
    Loads/tile: ACT 1650 / DVE 1850 / Pool 1612 vs PE 1712ns.
    Scores ship as fp8 [N,4096] exactly like the v1 baseline (same
    scales, same host rescore); sc DMAs are pair-batched.
    """
    import concourse.bass as bass
    import concourse.mybir as mybir
    from concourse import bacc
    from concourse.tile import TileContext

    dt = mybir.dt
    f8 = dt.float8e4

    nc = bacc.Bacc("TRN2", target_bir_lowering=False, debug=False)

    d_x8 = nc.dram_tensor("x8", [P, NT // 4, 4, NCP, 2, P], f8, kind="ExternalInput").ap()
    d_c8 = nc.dram_tensor("c8", [P, NKT, NCP, 2, 512], f8, kind="ExternalInput").ap()
    # [pair, p, t, col] -> host transposes to [N, K]
    d_sc = nc.dram_tensor("sc", [NT // 2, P, 2, K], f8, kind="ExternalOutput").ap()

    LOOKAHEAD = 1
    with TileContext(nc) as tc:
        with (
            tc.tile_pool(name="const", bufs=1) as cpool,
            tc.tile_pool(name="xt", bufs=LOOKAHEAD + 2) as xtp,
            tc.tile_pool(name="score", bufs=4) as scp,
            tc.tile_pool(name="psum", bufs=1, space="PSUM") as psp,
        ):
            x8_tiles = {}

            def fetch_x8(ip):
                if ip < NT // 4:
                    t = xtp.tile([P, 4, NCP, 2, P], f8, tag="x8")
                    nc.sync.dma_start(t[:], d_x8[:, ip, :, :, :, :])
                    x8_tiles[ip] = t

            fetch_x8(0)
            c8a = cpool.tile([P, 4, NCP, 2, 512], f8, tag="c8a", name="c8a")
            nc.scalar.dma_start(c8a[:], d_c8[:, 0:4, :, :, :])
            c8b = cpool.tile([P, 4, NCP, 2, 512], f8, tag="c8b", name="c8b")
            nc.scalar.dma_start(c8b[:], d_c8[:, 4:8, :, :, :])
            c8_tiles = [
                (c8a[:, kt] if kt < 4 else c8b[:, kt - 4]) for kt in range(NKT)
            ]
            fetch_x8(1)
            fetch_x8(2)

            # (tag, n_ktiles, consumer)
            SEGS = [
                ("e0", 1, "P"), ("e1", 1, "V"), ("q23", 2, "A"),
                ("q45", 2, "V"), ("e6", 1, "P"), ("e7", 1, "A"),
            ]

            for i in range(NT):
                if i % 4 == 0:
                    fetch_x8(i // 4 + LOOKAHEAD + 1)
                x8_quad = x8_tiles[i // 4]
                x8_t = x8_quad[:, i % 4]
                if i % 4 == 3:
                    x8_tiles.pop(i // 4)

                if i % 2 == 0:
                    sc2 = scp.tile([P, 2, K], f8, tag="sc")
                sc_t = sc2[:, i % 2]

                kt = 0
                for tag, nk, eng in SEGS:
                    ph = psp.tile([P, nk * 512], dt.float32, tag=tag, name=tag)
                    for s in range(nk):
                        for cp in range(NCP):
                            nc.tensor.matmul(
                                ph[:, s * 512:(s + 1) * 512],
                                lhsT=x8_t[:, cp, :, :],
                                rhs=c8_tiles[kt + s][:, cp, :, :],
                                start=(cp == 0), stop=(cp == NCP - 1),
                                perf_mode=mybir.MatmulPerfMode.DoubleRow,
                            )
                    lo = kt * 512
                    dst = sc_t[:, lo:lo + nk * 512]
                    if eng == "A":
                        nc.scalar.activation(
                            dst, ph[:], mybir.ActivationFunctionType.Identity,
                            scale=SCORE_SCALE)
                    elif eng == "V":
                        nc.vector.tensor_scalar_mul(dst, ph[:], SCORE_SCALE)
                    else:
                        nc.gpsimd.tensor_scalar_mul(dst, ph[:], SCORE_SCALE)
                    kt += nk

                if i % 2 == 1:
                    nc.sync.dma_start(d_sc[i // 2], sc2[:])

    nc.compile()
    return nc


def _build_fp8r3():
    """HW-legal: only ACT and DVE may read PSUM (GPSIMD cannot; and any
    instruction may read at most one PSUM operand).

    Per tile: 4 quarter segments [P,1024] (2 banks each), 2 matmuls per
    segment (1024-col DoubleRow, quad codebook layout), consumers:
      pos0-1 quad0 ACT raw fp8 evict  1038ns   codes 0..1023
      pos2-3 quad1 DVE G4 reduce fp16 1192ns   codes 1024..2047
      pos4-5 quad2 ACT raw fp8 evict  1038ns   codes 2048..3071
      pos6-7 quad3 DVE G4 reduce fp16 1192ns   codes 3072..4095
    Loads/tile: ACT 2076 / DVE 2384 (the legal 2-engine PSUM-drain
    bound); PE 1712.  Ships 2 KB raw fp8 + 1 KB fp16 G4 maxes per
    token (12 MB/core).
    """
    import concourse.bass as bass
    import concourse.mybir as mybir
    from concourse import bacc
    from concourse.tile import TileContext

    dt = mybir.dt
    f8 = dt.float8e4
    f16 = dt.float16

    nc = bacc.Bacc("TRN2", target_bir_lowering=False, debug=False)

    d_x8 = nc.dram_tensor("x8", [P, NT // 4, 4, NCP, 2, P], f8, kind="ExternalInput").ap()
    d_c8 = nc.dram_tensor("c8", [P, 4, NCP, 2, 1024], f8, kind="ExternalInput").ap()
    d_raw = nc.dram_tensor("raw", [NT // 4, P, 8704], f8, kind="ExternalOutput").ap()
    d_red = nc.dram_tensor("red", [NT // 4, P, 1920], f16, kind="ExternalOutput").ap()

    LOOKAHEAD = 1
    with TileContext(nc) as tc:
        with (
            tc.tile_pool(name="const", bufs=1) as cpool,
            tc.tile_pool(name="xt", bufs=LOOKAHEAD + 2) as xtp,
            tc.tile_pool(name="raws", bufs=4) as rwp,
            tc.tile_pool(name="reds", bufs=4) as rdp,
            tc.tile_pool(name="psum", bufs=1, space="PSUM") as psp,
        ):
            x8_tiles = {}

            def fetch_x8(ip):
                if ip < NT // 4:
                    t = xtp.tile([P, 4, NCP, 2, P], f8, tag="x8")
                    nc.sync.dma_start(t[:], d_x8[:, ip, :, :, :, :])
                    x8_tiles[ip] = t

            # DVE reduces quads 0,2; ACT evicts quads 1,3.  Fetch the
            # codebook in DVE-first order (0,2,1,3): DVE is the binding
            # engine, so its pipeline must start as early as possible.
            c8t = [None] * 4
            for qt in (0, 2, 1, 3):
                tq = cpool.tile([P, NCP, 2, 1024], f8, tag=f"c8q{qt}",
                                name=f"c8q{qt}")
                if qt in (0, 2):
                    # half-quad DMAs: the first matmul only needs cols
                    # 0:512, so it can start one transfer earlier
                    nc.sync.dma_start(tq[:, :, :, 0:512],
                                      d_c8[:, qt, :, :, 0:512])
                    if qt == 0:
                        fetch_x8(0)
                    nc.sync.dma_start(tq[:, :, :, 512:1024],
                                      d_c8[:, qt, :, :, 512:1024])
                else:
                    nc.sync.dma_start(tq[:], d_c8[:, qt, :, :, :])
                c8t[qt] = tq
            fetch_x8(1)
            fetch_x8(2)

            for i in range(NT):
                if i % 4 == 0:
                    fetch_x8(i // 4 + LOOKAHEAD + 1)
                x8_quad = x8_tiles[i // 4]
                x8_t = x8_quad[:, i % 4]
                if i % 4 == 3:
                    x8_tiles.pop(i // 4)

                if i % 4 == 0:
                    raw4 = rwp.tile([P, 8704], f8, tag="raw")
                    red4 = rdp.tile([P, 1920], f16, tag="red")
                conv = (i % 4 == 3)
                raw_t = raw4[:, (i % 4) * 2048:
                             (i % 4) * 2048 + (2560 if conv else 2048)]
                red_t = red4[:, (i % 4) * 512:
                             (i % 4) * 512 + (384 if conv else 512)]

                for seg in range(4):
                    dve = seg % 2 == 0
                    if dve:
                        ph = psp.tile([P, 256, 4], dt.float32, tag=f"g{seg}",
                                      name=f"g{seg}")
                    else:
                        ph = psp.tile([P, 1024], dt.float32, tag=f"g{seg}",
                                      name=f"g{seg}")
                    # fp8 DR matmuls are limited to 512 moving cols
                    for s in range(2):
                        dst = (ph[:, s * 128:(s + 1) * 128, :] if dve
                               else ph[:, s * 512:(s + 1) * 512])
                        for cp in range(NCP):
                            nc.tensor.matmul(
                                dst,
                                lhsT=x8_t[:, cp, :, :],
                                rhs=c8t[seg][:, cp, :, s * 512:(s + 1) * 512],
                                start=(cp == 0), stop=(cp == NCP - 1),
                                perf_mode=mybir.MatmulPerfMode.DoubleRow,
                            )
                    if seg == 2 and conv:
                        # converted tile: ACT raw-evicts the 2nd half of
                        # quad2 (free sizes match: [128,4] view = 512),
                        # DVE G4-reduces only the 1st half
                        nc.scalar.activation(
                            raw_t[:, 2048:2560], ph[:, 128:256, :],
                            mybir.ActivationFunctionType.Identity,
                            scale=SCORE_SCALE)
                        nc.vector.tensor_reduce(
                            red_t[:, 256:384], ph[:, 0:128, :],
                            axis=mybir.AxisListType.X, op=mybir.AluOpType.max)
                    elif dve:
                        off = (seg // 2) * 256
                        nc.vector.tensor_reduce(
                            red_t[:, off:off + 256], ph[:],
                            axis=mybir.AxisListType.X, op=mybir.AluOpType.max)
                    else:
                        off = (seg // 2) * 1024
                        nc.scalar.activation(
                            raw_t[:, off:off + 1024], ph[:],
                            mybir.ActivationFunctionType.Identity,
                            scale=SCORE_SCALE)

                if i >= NT - 4:
                    lo_r = (i % 4) * 2048
                    hi_r = lo_r + (2560 if conv else 2048)
                    lo_d = (i % 4) * 512
                    hi_d = lo_d + (384 if conv else 512)
                    nc.sync.dma_start(d_red[i // 4][:, lo_d:hi_d], red_t)
                    nc.sync.dma_start(d_raw[i // 4][:, lo_r:hi_r], raw_t)
                elif i % 4 == 3:
                    nc.sync.dma_start(d_raw[i // 4], raw4[:])
                    nc.sync.dma_start(d_red[i // 4], red4[:])

    nc.compile()
    return nc




def _build_fp8r2():
    """HW-legal: only ACT and DVE may read PSUM (GPSIMD cannot; and any
    instruction may read at most one PSUM operand).

    Per tile: 4 quarter segments [P,1024] (2 banks each), 2 matmuls per
    segment (1024-col DoubleRow, quad codebook layout), consumers:
      pos0-1 quad0 ACT raw fp8 evict  1038ns   codes 0..1023
      pos2-3 quad1 DVE G4 reduce fp16 1192ns   codes 1024..2047
      pos4-5 quad2 ACT raw fp8 evict  1038ns   codes 2048..3071
      pos6-7 quad3 DVE G4 reduce fp16 1192ns   codes 3072..4095
    Loads/tile: ACT 2076 / DVE 2384 (the legal 2-engine PSUM-drain
    bound); PE 1712.  Ships 2 KB raw fp8 + 1 KB fp16 G4 maxes per
    token (12 MB/core).
    """
    import concourse.bass as bass
    import concourse.mybir as mybir
    from concourse import bacc
    from concourse.tile import TileContext

    dt = mybir.dt
    f8 = dt.float8e4
    f16 = dt.float16

    nc = bacc.Bacc("TRN2", target_bir_lowering=False, debug=False)

    d_x8 = nc.dram_tensor("x8", [P, NT // 4, 4, NCP, 2, P], f8, kind="ExternalInput").ap()
    d_c8 = nc.dram_tensor("c8", [P, 4, NCP, 2, 1024], f8, kind="ExternalInput").ap()
    d_raw = nc.dram_tensor("raw", [NT // 4, P, 4, 2048], f8, kind="ExternalOutput").ap()
    d_red = nc.dram_tensor("red", [NT // 4, P, 4, 512], f16, kind="ExternalOutput").ap()

    LOOKAHEAD = 1
    with TileContext(nc) as tc:
        with (
            tc.tile_pool(name="const", bufs=1) as cpool,
            tc.tile_pool(name="xt", bufs=LOOKAHEAD + 2) as xtp,
            tc.tile_pool(name="raws", bufs=4) as rwp,
            tc.tile_pool(name="reds", bufs=4) as rdp,
            tc.tile_pool(name="psum", bufs=1, space="PSUM") as psp,
        ):
            x8_tiles = {}

            def fetch_x8(ip):
                if ip < NT // 4:
                    t = xtp.tile([P, 4, NCP, 2, P], f8, tag="x8")
                    nc.sync.dma_start(t[:], d_x8[:, ip, :, :, :, :])
                    x8_tiles[ip] = t

            # DVE reduces quads 0,2; ACT evicts quads 1,3.  Fetch the
            # codebook in DVE-first order (0,2,1,3): DVE is the binding
            # engine, so its pipeline must start as early as possible.
            c8t = [None] * 4
            for qt in (0, 2, 1, 3):
                tq = cpool.tile([P, NCP, 2, 1024], f8, tag=f"c8q{qt}",
                                name=f"c8q{qt}")
                if qt in (0, 2):
                    # half-quad DMAs: the first matmul only needs cols
                    # 0:512, so it can start one transfer earlier
                    nc.sync.dma_start(tq[:, :, :, 0:512],
                                      d_c8[:, qt, :, :, 0:512])
                    if qt == 0:
                        fetch_x8(0)
                    nc.sync.dma_start(tq[:, :, :, 512:1024],
                                      d_c8[:, qt, :, :, 512:1024])
                else:
                    nc.sync.dma_start(tq[:], d_c8[:, qt, :, :, :])
                c8t[qt] = tq
            fetch_x8(1)
            fetch_x8(2)

            for i in range(NT):
                if i % 4 == 0:
                    fetch_x8(i // 4 + LOOKAHEAD + 1)
                x8_quad = x8_tiles[i // 4]
                x8_t = x8_quad[:, i % 4]
                if i % 4 == 3:
                    x8_tiles.pop(i // 4)

                if i % 4 == 0:
                    raw4 = rwp.tile([P, 4, 2048], f8, tag="raw")
                    red4 = rdp.tile([P, 4, 512], f16, tag="red")
                raw_t = raw4[:, i % 4]
                red_t = red4[:, i % 4]

                for seg in range(4):
                    dve = seg % 2 == 0
                    if dve:
                        ph = psp.tile([P, 256, 4], dt.float32, tag=f"g{seg}",
                                      name=f"g{seg}")
                    else:
                        ph = psp.tile([P, 1024], dt.float32, tag=f"g{seg}",
                                      name=f"g{seg}")
                    # fp8 DR matmuls are limited to 512 moving cols
                    for s in range(2):
                        dst = (ph[:, s * 128:(s + 1) * 128, :] if dve
                               else ph[:, s * 512:(s + 1) * 512])
                        for cp in range(NCP):
                            nc.tensor.matmul(
                                dst,
                                lhsT=x8_t[:, cp, :, :],
                                rhs=c8t[seg][:, cp, :, s * 512:(s + 1) * 512],
                                start=(cp == 0), stop=(cp == NCP - 1),
                                perf_mode=mybir.MatmulPerfMode.DoubleRow,
                            )
                    if dve:
                        off = (seg // 2) * 256
                        nc.vector.tensor_reduce(
                            red_t[:, off:off + 256], ph[:],
                            axis=mybir.AxisListType.X, op=mybir.AluOpType.max)
                    else:
                        off = (seg // 2) * 1024
                        nc.scalar.activation(
                            raw_t[:, off:off + 1024], ph[:],
                            mybir.ActivationFunctionType.Identity,
                            scale=SCORE_SCALE)

                if i >= NT - 4:
                    nc.sync.dma_start(d_red[i // 4][:, i % 4], red_t)
                    nc.sync.dma_start(d_raw[i // 4][:, i % 4], raw_t)
                elif i % 4 == 3:
                    nc.sync.dma_start(d_raw[i // 4], raw4[:])
                    nc.sync.dma_start(d_red[i // 4], red4[:])

    nc.compile()
    return nc


def _build_gmax7():
    """All-eighths: 8 one-bank PSUM segments, one consumer each.

    Per-segment cycle bound = 213 (MMs) + ~470 (drain+sem+SEQ) + dur +
    ~150 (sem back) stays under the tile period for every segment;
    engine loads/tile: ACT 3x612=1836, DVE 3x658=1974, Pool 2x806=1612.
      pos0 kt0 ACT raw | pos1 kt1 DVE G4 | pos2 kt2 Pool raw
      pos3 kt3 ACT raw | pos4 kt4 DVE G4 | pos5 kt5 Pool raw
      pos6 kt6 ACT raw | pos7 kt7 DVE G4
    Ships 2560 B raw fp8 + 384 fp16 G4 maxes per token (13 MB/core).
    """
    import concourse.bass as bass
    import concourse.mybir as mybir
    from concourse import bacc
    from concourse.tile import TileContext

    dt = mybir.dt
    f8 = dt.float8e4
    f16 = dt.float16

    nc = bacc.Bacc("TRN2", target_bir_lowering=False, debug=False)

    d_x8 = nc.dram_tensor("x8", [P, NT // 4, 4, NCP, 2, P], f8, kind="ExternalInput").ap()
    d_c8 = nc.dram_tensor("c8", [P, NKT, NCP, 2, 512], f8, kind="ExternalInput").ap()
    d_raw = nc.dram_tensor("raw", [NT // 4, P, 4, 2560], f8, kind="ExternalOutput").ap()
    d_red = nc.dram_tensor("red", [NT // 4, P, 4, 384], f16, kind="ExternalOutput").ap()

    # (kt, consumer, raw_off_or_red_off)
    SEGS = [
        (0, "A", 0), (1, "V", 0), (2, "P", 512), (3, "A", 1024),
        (4, "V", 128), (5, "P", 1536), (6, "A", 2048), (7, "V", 256),
    ]

    LOOKAHEAD = 1
    with TileContext(nc) as tc:
        with (
            tc.tile_pool(name="const", bufs=1) as cpool,
            tc.tile_pool(name="xt", bufs=LOOKAHEAD + 2) as xtp,
            tc.tile_pool(name="raws", bufs=4) as rwp,
            tc.tile_pool(name="reds", bufs=4) as rdp,
            tc.tile_pool(name="psum", bufs=1, space="PSUM") as psp,
        ):
            x8_tiles = {}

            def fetch_x8(ip):
                if ip < NT // 4:
                    t = xtp.tile([P, 4, NCP, 2, P], f8, tag="x8")
                    nc.sync.dma_start(t[:], d_x8[:, ip, :, :, :, :])
                    x8_tiles[ip] = t

            fetch_x8(0)
            c8a = cpool.tile([P, 4, NCP, 2, 512], f8, tag="c8a", name="c8a")
            nc.scalar.dma_start(c8a[:], d_c8[:, 0:4, :, :, :])
            c8b = cpool.tile([P, 4, NCP, 2, 512], f8, tag="c8b", name="c8b")
            nc.scalar.dma_start(c8b[:], d_c8[:, 4:8, :, :, :])
            c8_tiles = [
                (c8a[:, kt] if kt < 4 else c8b[:, kt - 4]) for kt in range(NKT)
            ]
            fetch_x8(1)
            fetch_x8(2)

            for i in range(NT):
                if i % 4 == 0:
                    fetch_x8(i // 4 + LOOKAHEAD + 1)
                x8_quad = x8_tiles[i // 4]
                x8_t = x8_quad[:, i % 4]
                if i % 4 == 3:
                    x8_tiles.pop(i // 4)

                if i % 4 == 0:
                    raw4 = rwp.tile([P, 4, 2560], f8, tag="raw")
                    red4 = rdp.tile([P, 4, 384], f16, tag="red")
                raw_t = raw4[:, i % 4]
                red_t = red4[:, i % 4]

                for kt, eng, off in SEGS:
                    if eng == "V":
                        ph = psp.tile([P, 128, 4], dt.float32, tag=f"s{kt}",
                                      name=f"s{kt}")
                    else:
                        ph = psp.tile([P, 512], dt.float32, tag=f"s{kt}",
                                      name=f"s{kt}")
                    for cp in range(NCP):
                        nc.tensor.matmul(
                            ph[:],
                            lhsT=x8_t[:, cp, :, :],
                            rhs=c8_tiles[kt][:, cp, :, :],
                            start=(cp == 0), stop=(cp == NCP - 1),
                            perf_mode=mybir.MatmulPerfMode.DoubleRow,
                        )
                    if eng == "A":
                        nc.scalar.activation(
                            raw_t[:, off:off + 512], ph[:],
                            mybir.ActivationFunctionType.Identity,
                            scale=SCORE_SCALE)
                    elif eng == "P":
                        nc.gpsimd.tensor_scalar_mul(
                            raw_t[:, off:off + 512], ph[:], SCORE_SCALE)
                    else:
                        nc.vector.tensor_reduce(
                            red_t[:, off:off + 128], ph[:],
                            axis=mybir.AxisListType.X, op=mybir.AluOpType.max)

                if i % 4 == 3:
                    nc.sync.dma_start(d_raw[i // 4], raw4[:])
                    nc.sync.dma_start(d_red[i // 4], red4[:])

    nc.compile()
    return nc


def _build_gmax6():
    """gmax5 with 1024-col DoubleRow matmuls (12 MMs/tile vs 16).

    Codebook is stored in the quad layout [P, 4, NCP, 2, 1024]; an
    eighth k-tile is the 512-col half of a quad slice, so one layout
    serves both the 1024-col quarter matmuls and the 512-col eighth
    matmuls.  Consumers are as in gmax5 (one PSUM operand each):
      e0 Pool raw | q12 DVE G4-reduce | q34 ACT raw | e5 DVE G2-reduce
      | e6 Pool raw | e7 ACT raw.
    """
    import concourse.bass as bass
    import concourse.mybir as mybir
    from concourse import bacc
    from concourse.tile import TileContext

    dt = mybir.dt
    f8 = dt.float8e4
    f16 = dt.float16

    nc = bacc.Bacc("TRN2", target_bir_lowering=False, debug=False)

    d_x8 = nc.dram_tensor("x8", [P, NT // 4, 4, NCP, 2, P], f8, kind="ExternalInput").ap()
    d_c8 = nc.dram_tensor("c8", [P, 4, NCP, 2, 1024], f8, kind="ExternalInput").ap()
    d_raw = nc.dram_tensor("raw", [NT // 4, P, 4, 2560], f8, kind="ExternalOutput").ap()
    d_red = nc.dram_tensor("red", [NT // 4, P, 4, 512], f16, kind="ExternalOutput").ap()

    LOOKAHEAD = 1
    with TileContext(nc) as tc:
        with (
            tc.tile_pool(name="const", bufs=1) as cpool,
            tc.tile_pool(name="xt", bufs=LOOKAHEAD + 2) as xtp,
            tc.tile_pool(name="raws", bufs=4) as rwp,
            tc.tile_pool(name="reds", bufs=4) as rdp,
            tc.tile_pool(name="psum", bufs=1, space="PSUM") as psp,
        ):
            x8_tiles = {}

            def fetch_x8(ip):
                if ip < NT // 4:
                    t = xtp.tile([P, 4, NCP, 2, P], f8, tag="x8")
                    nc.sync.dma_start(t[:], d_x8[:, ip, :, :, :, :])
                    x8_tiles[ip] = t

            # codebook in 4 single-quad DMAs: quad0 lands first so the
            # first matmuls can start ~2us earlier than a bulk fetch
            c8t = []
            for qt in range(4):
                tq = cpool.tile([P, NCP, 2, 1024], f8, tag=f"c8q{qt}",
                                name=f"c8q{qt}")
                nc.scalar.dma_start(tq[:], d_c8[:, qt, :, :, :])
                c8t.append(tq)
                if qt == 0:
                    fetch_x8(0)

            def c8q(qt):   # quad qt: [P, NCP, 2, 1024]
                return c8t[qt][:]

            def c8e(kt):   # eighth kt: [P, NCP, 2, 512]
                q = c8q(kt // 2)
                h = (kt % 2) * 512
                return q[:, :, :, h:h + 512]

            fetch_x8(1)
            fetch_x8(2)

            def mm_q(dst1024, x8_t, qt):
                for cp in range(NCP):
                    nc.tensor.matmul(
                        dst1024,
                        lhsT=x8_t[:, cp, :, :],
                        rhs=c8q(qt)[:, cp, :, :],
                        start=(cp == 0), stop=(cp == NCP - 1),
                        perf_mode=mybir.MatmulPerfMode.DoubleRow,
                    )

            def mm_e(dst512, x8_t, kt):
                for cp in range(NCP):
                    nc.tensor.matmul(
                        dst512,
                        lhsT=x8_t[:, cp, :, :],
                        rhs=c8e(kt)[:, cp, :, :],
                        start=(cp == 0), stop=(cp == NCP - 1),
                        perf_mode=mybir.MatmulPerfMode.DoubleRow,
                    )

            for i in range(NT):
                if i % 4 == 0:
                    fetch_x8(i // 4 + LOOKAHEAD + 1)
                x8_quad = x8_tiles[i // 4]
                x8_t = x8_quad[:, i % 4]
                if i % 4 == 3:
                    x8_tiles.pop(i // 4)

                if i % 4 == 0:
                    raw4 = rwp.tile([P, 4, 2560], f8, tag="raw")
                    red4 = rdp.tile([P, 4, 512], f16, tag="red")
                raw_t = raw4[:, i % 4]
                red_t = red4[:, i % 4]

                # pos0: kt0 -> Pool raw evict
                e0 = psp.tile([P, 512], dt.float32, tag="e0", name="e0")
                mm_e(e0[:], x8_t, 0)
                nc.gpsimd.tensor_scalar_mul(raw_t[:, 0:512], e0[:], SCORE_SCALE)
                # pos1-2: kt1,kt2 = quad1... quads are [kt0,kt1],[kt2,kt3],...
                # use quad cols directly: q12 covers codes 1024..2047 (quad 1)
                q12 = psp.tile([P, 256, 4], dt.float32, tag="q12", name="q12")
                mm_q(q12[:], x8_t, 1)
                nc.vector.tensor_reduce(
                    red_t[:, 256:512], q12[:], axis=mybir.AxisListType.X,
                    op=mybir.AluOpType.max)
                # pos3-4: quad 2 = codes 2048..3071 -> ACT raw evict
                q34 = psp.tile([P, 1024], dt.float32, tag="q34", name="q34")
                mm_q(q34[:], x8_t, 2)
                nc.scalar.activation(
                    raw_t[:, 512:1536], q34[:],
                    mybir.ActivationFunctionType.Identity, scale=SCORE_SCALE)
                # pos5: kt1 (codes 512..1023) -> DVE reduce G2
                e5 = psp.tile([P, 256, 2], dt.float32, tag="e5", name="e5")
                mm_e(e5[:], x8_t, 1)
                nc.vector.tensor_reduce(
                    red_t[:, 0:256], e5[:], axis=mybir.AxisListType.X,
                    op=mybir.AluOpType.max)
                # pos6: kt6 -> Pool raw evict
                e6 = psp.tile([P, 512], dt.float32, tag="e6", name="e6")
                mm_e(e6[:], x8_t, 6)
                nc.gpsimd.tensor_scalar_mul(raw_t[:, 1536:2048], e6[:], SCORE_SCALE)
                # pos7: kt7 -> ACT raw evict
                e7 = psp.tile([P, 512], dt.float32, tag="e7", name="e7")
                mm_e(e7[:], x8_t, 7)
                nc.scalar.activation(
                    raw_t[:, 2048:2560], e7[:],
                    mybir.ActivationFunctionType.Identity, scale=SCORE_SCALE)

                if i == NT - 4:
                    # final quad: per-tile DMAs so the tail isn't one big
                    # 3.6us transfer serialized after the last consumer
                    nc.sync.dma_start(d_red[i // 4][:, i % 4], red_t)
                    nc.sync.dma_start(d_raw[i // 4][:, i % 4], raw_t)
                elif i > NT - 4:
                    nc.sync.dma_start(d_raw[i // 4][:, i % 4], raw_t)
                    nc.sync.dma_start(d_red[i // 4][:, i % 4], red_t)
                elif i % 4 == 3:
                    nc.sync.dma_start(d_raw[i // 4], raw4[:])
                    nc.sync.dma_start(d_red[i // 4], red4[:])

    nc.compile()
    return nc


def _build_gmax5():
    """HW-legal mixed raw/group-max kernel.

    Legal PSUM consumers only (one PSUM operand per instruction):
      pos0   e0  [512]  Pool tensor_scalar evict -> fp8 raw      806ns
      pos1   e1  [512]  DVE  tensor_reduce G2    -> fp16 [256]   658ns
      pos2-3 q23 [1024] ACT  activation evict    -> fp8 raw     1038ns
      pos4-5 q45 [1024] DVE  tensor_reduce G4    -> fp16 [256]  1192ns
      pos6   e6  [512]  Pool tensor_scalar evict -> fp8 raw      806ns
      pos7   e7  [512]  ACT  activation evict    -> fp8 raw      612ns
    Loads/tile: ACT 1650 / DVE 1850 / Pool 1612; PE 1712ns.
    Ships per token: 2560 B raw fp8 + 256+256 fp16 group maxes
    (3.5 KB -> 14 MB/core).  Host: margin candidates from raw cols
    (codes kt0, kt2, kt3, kt6, kt7) plus expanded groups (kt1 pairs,
    kt4-5 quads), exact rescore as in the baseline.
    """
    import concourse.bass as bass
    import concourse.mybir as mybir
    from concourse import bacc
    from concourse.tile import TileContext

    dt = mybir.dt
    f8 = dt.float8e4
    f16 = dt.float16

    nc = bacc.Bacc("TRN2", target_bir_lowering=False, debug=False)

    d_x8 = nc.dram_tensor("x8", [P, NT // 4, 4, NCP, 2, P], f8, kind="ExternalInput").ap()
    d_c8 = nc.dram_tensor("c8", [P, NKT, NCP, 2, 512], f8, kind="ExternalInput").ap()
    # raw fp8 segments [kt0 | kt2 kt3 | kt6 | kt7], batched 4 tiles/DMA
    d_raw = nc.dram_tensor("raw", [NT // 4, P, 4, 2560], f8, kind="ExternalOutput").ap()
    # fp16 group maxes [kt1-G2 (256) | kt45-G4 (256)]
    d_red = nc.dram_tensor("red", [NT // 4, P, 4, 512], f16, kind="ExternalOutput").ap()

    LOOKAHEAD = 1
    with TileContext(nc) as tc:
        with (
            tc.tile_pool(name="const", bufs=1) as cpool,
            tc.tile_pool(name="xt", bufs=LOOKAHEAD + 2) as xtp,
            tc.tile_pool(name="raws", bufs=3) as rwp,
            tc.tile_pool(name="reds", bufs=3) as rdp,
            tc.tile_pool(name="psum", bufs=1, space="PSUM") as psp,
        ):
            x8_tiles = {}

            def fetch_x8(ip):
                if ip < NT // 4:
                    t = xtp.tile([P, 4, NCP, 2, P], f8, tag="x8")
                    nc.sync.dma_start(t[:], d_x8[:, ip, :, :, :, :])
                    x8_tiles[ip] = t

            fetch_x8(0)
            c8a = cpool.tile([P, 4, NCP, 2, 512], f8, tag="c8a", name="c8a")
            nc.scalar.dma_start(c8a[:], d_c8[:, 0:4, :, :, :])
            c8b = cpool.tile([P, 4, NCP, 2, 512], f8, tag="c8b", name="c8b")
            nc.scalar.dma_start(c8b[:], d_c8[:, 4:8, :, :, :])
            c8_tiles = [
                (c8a[:, kt] if kt < 4 else c8b[:, kt - 4]) for kt in range(NKT)
            ]
            fetch_x8(1)
            fetch_x8(2)

            def mms(ph_slices, x8_t, kts):
                # ph_slices: list of [P,512] psum dsts, one per ktile
                for dst, kt in zip(ph_slices, kts):
                    for cp in range(NCP):
                        nc.tensor.matmul(
                            dst,
                            lhsT=x8_t[:, cp, :, :],
                            rhs=c8_tiles[kt][:, cp, :, :],
                            start=(cp == 0), stop=(cp == NCP - 1),
                            perf_mode=mybir.MatmulPerfMode.DoubleRow,
                        )

            for i in range(NT):
                if i % 4 == 0:
                    fetch_x8(i // 4 + LOOKAHEAD + 1)
                x8_quad = x8_tiles[i // 4]
                x8_t = x8_quad[:, i % 4]
                if i % 4 == 3:
                    x8_tiles.pop(i // 4)

                if i % 4 == 0:
                    raw4 = rwp.tile([P, 4, 2560], f8, tag="raw")
                    red4 = rdp.tile([P, 4, 512], f16, tag="red")
                raw_t = raw4[:, i % 4]
                red_t = red4[:, i % 4]

                # pos0: kt0 -> Pool raw evict
                e0 = psp.tile([P, 512], dt.float32, tag="e0", name="e0")
                mms([e0[:]], x8_t, [0])
                nc.gpsimd.tensor_scalar_mul(raw_t[:, 0:512], e0[:], SCORE_SCALE)
                # pos1-2: kt1,kt2 -> DVE reduce G4 (early so DVE's big op
                # doesn't queue behind its small one)
                q12 = psp.tile([P, 256, 4], dt.float32, tag="q12", name="q12")
                mms([q12[:, 0:128, :], q12[:, 128:256, :]], x8_t, [1, 2])
                nc.vector.tensor_reduce(
                    red_t[:, 256:512], q12[:], axis=mybir.AxisListType.X,
                    op=mybir.AluOpType.max)
                # pos3-4: kt3,kt4 -> ACT raw evict
                q34 = psp.tile([P, 1024], dt.float32, tag="q34", name="q34")
                mms([q34[:, 0:512], q34[:, 512:1024]], x8_t, [3, 4])
                nc.scalar.activation(
                    raw_t[:, 512:1536], q34[:],
                    mybir.ActivationFunctionType.Identity, scale=SCORE_SCALE)
                # pos5: kt5 -> DVE reduce G2
                e5 = psp.tile([P, 256, 2], dt.float32, tag="e5", name="e5")
                mms([e5[:]], x8_t, [5])
                nc.vector.tensor_reduce(
                    red_t[:, 0:256], e5[:], axis=mybir.AxisListType.X,
                    op=mybir.AluOpType.max)
                # pos6: kt6 -> Pool raw evict
                e6 = psp.tile([P, 512], dt.float32, tag="e6", name="e6")
                mms([e6[:]], x8_t, [6])
                nc.gpsimd.tensor_scalar_mul(raw_t[:, 1536:2048], e6[:], SCORE_SCALE)
                # pos7: kt7 -> ACT raw evict
                e7 = psp.tile([P, 512], dt.float32, tag="e7", name="e7")
                mms([e7[:]], x8_t, [7])
                nc.scalar.activation(
                    raw_t[:, 2048:2560], e7[:],
                    mybir.ActivationFunctionType.Identity, scale=SCORE_SCALE)

                if i % 4 == 3:
                    nc.sync.dma_start(d_raw[i // 4], raw4[:])
                    nc.sync.dma_start(d_red[i // 4], red4[:])

    nc.compile()
    return nc


def _build_gmax4():
    """On-device group-max (G=4) reduction of the fp8 score matrix.

    Per 128-token tile: 16 DR matmuls fill 8 one-bank PSUM eighths
    ([P,512] f32, ring of 8).  A max fold tree reduces the 4096 raw
    scores to 1024 group-maxes (stride-512 groups of 4) in fp16:
      m01 = Pool max(e0,e1)   806ns   (psum f32 in)
      m23 = DVE  max(e2,e3)   658ns
      m45 = Pool max(e4,e5)   806ns
      ev6 = ACT  evict(e6)    612ns \\  fp16 copies
      ev7 = ACT  evict(e7)    612ns /
      m67 = DVE  max(ev6,ev7) 327ns   (fp16 sbuf, 2x mode)
      g0  = DVE  max(m01,m23) 327ns -> gm[:,0,:]
      g1  = DVE  max(m45,m67) 327ns -> gm[:,1,:]
    Per-tile busy: ACT 1224 / DVE 1639 / Pool 1612 < PE 1707ns.
    Ships [N,1024] fp16 raw psum-scale maxes (8MB/core vs 16MB for
    full fp8 scores); host rescores groups within margin exactly.
    """
    import concourse.bass as bass
    import concourse.mybir as mybir
    from concourse import bacc
    from concourse.tile import TileContext

    dt = mybir.dt
    f8 = dt.float8e4
    f16 = dt.float16

    nc = bacc.Bacc("TRN2", target_bir_lowering=False, debug=False)

    d_x8 = nc.dram_tensor("x8", [P, NT // 4, 4, NCP, 2, P], f8, kind="ExternalInput").ap()
    d_c8 = nc.dram_tensor("c8", [P, NKT, NCP, 2, 512], f8, kind="ExternalInput").ap()
    # [pair, p, t, col]: host reshapes to [N, 1024] via transpose(0,2,1,3)
    d_gm = nc.dram_tensor("gm", [NT // 2, P, 2, 1024], f16, kind="ExternalOutput").ap()

    LOOKAHEAD = 1  # x8 quad-tile prefetch depth
    with TileContext(nc) as tc:
        with (
            tc.tile_pool(name="const", bufs=1) as cpool,
            tc.tile_pool(name="xt", bufs=LOOKAHEAD + 2) as xtp,
            tc.tile_pool(name="fold", bufs=3) as fpool,
            tc.tile_pool(name="gmx", bufs=3) as gpool,
            tc.tile_pool(name="psum", bufs=8, space="PSUM") as psp,
        ):
            x8_tiles = {}

            def fetch_x8(ip):  # fetch quad ip (tiles 4*ip .. 4*ip+3)
                if ip < NT // 4:
                    t = xtp.tile([P, 4, NCP, 2, P], f8, tag="x8")
                    nc.sync.dma_start(t[:], d_x8[:, ip, :, :, :, :])
                    x8_tiles[ip] = t

            fetch_x8(0)
            # codebook in two 4-ktile DMAs issued from the (startup-idle)
            # ACT and DVE sequencers so they overlap the x8 issue on SP
            c8a = cpool.tile([P, 4, NCP, 2, 512], f8, tag="c8a", name="c8a")
            nc.scalar.dma_start(c8a[:], d_c8[:, 0:4, :, :, :])
            c8b = cpool.tile([P, 4, NCP, 2, 512], f8, tag="c8b", name="c8b")
            nc.scalar.dma_start(c8b[:], d_c8[:, 4:8, :, :, :])
            c8_tiles = [
                (c8a[:, kt] if kt < 4 else c8b[:, kt - 4]) for kt in range(NKT)
            ]
            fetch_x8(1)
            fetch_x8(2)

            for i in range(NT):
                if i % 4 == 0:
                    fetch_x8(i // 4 + LOOKAHEAD + 1)
                x8_quad = x8_tiles[i // 4]
                x8_t = x8_quad[:, i % 4]
                if i % 4 == 3:
                    x8_tiles.pop(i // 4)

                es = []
                for kt in range(NKT):
                    e = psp.tile([P, 512], dt.float32, tag="pe")
                    for cp in range(NCP):
                        nc.tensor.matmul(
                            e[:],
                            lhsT=x8_t[:, cp, :, :],
                            rhs=c8_tiles[kt][:, cp, :, :],
                            start=(cp == 0), stop=(cp == NCP - 1),
                            perf_mode=mybir.MatmulPerfMode.DoubleRow,
                        )
                    es.append(e)

                if i % 2 == 0:
                    gm2 = gpool.tile([P, 2, 1024], f16, tag="gm")
                gm = gm2[:, i % 2]
                m01 = fpool.tile([P, 512], f16, tag="m01", name="m01")
                m23 = fpool.tile([P, 512], f16, tag="m23", name="m23")
                m45 = fpool.tile([P, 512], f16, tag="m45", name="m45")
                ev6 = fpool.tile([P, 512], f16, tag="ev6", name="ev6")
                ev7 = fpool.tile([P, 512], f16, tag="ev7", name="ev7")
                nc.gpsimd.tensor_max(m01[:], es[0][:], es[1][:])
                nc.vector.tensor_max(m23[:], es[2][:], es[3][:])
                nc.gpsimd.tensor_max(m45[:], es[4][:], es[5][:])
                nc.scalar.activation(
                    ev6[:], es[6][:], mybir.ActivationFunctionType.Identity)
                nc.scalar.activation(
                    ev7[:], es[7][:], mybir.ActivationFunctionType.Identity)
                m67 = fpool.tile([P, 512], f16, tag="m67", name="m67")
                nc.vector.tensor_max(m67[:], ev6[:], ev7[:])
                nc.vector.tensor_max(gm[:, 0:512], m01[:], m23[:])
                nc.vector.tensor_max(gm[:, 512:1024], m45[:], m67[:])
                if i % 2 == 1:
                    nc.sync.dma_start(d_gm[i // 2], gm2[:])

    nc.compile()
    return nc


def _build_fp8ship():
    import concourse.bass as bass
    import concourse.mybir as mybir
    from concourse import bacc
    from concourse.tile import TileContext

    dt = mybir.dt
    f16 = dt.float16
    f8 = dt.float8e4

    nc = bacc.Bacc("TRN2", target_bir_lowering=False, debug=False)

    d_x8 = nc.dram_tensor("x8", [P, NT, NCP, 2, P], f8, kind="ExternalInput").ap()
    d_c8 = nc.dram_tensor("c8", [P, NKT, NCP, 2, 512], f8, kind="ExternalInput").ap()
    d_sc = nc.dram_tensor("sc", [N, K], f8, kind="ExternalOutput").ap()

    LOOKAHEAD = 3
    with TileContext(nc) as tc:
        with (
            tc.tile_pool(name="const", bufs=1) as cpool,
            tc.tile_pool(name="xt", bufs=LOOKAHEAD + 2) as xtp,
            tc.tile_pool(name="score", bufs=4) as scp,
            tc.tile_pool(name="psum", bufs=4, space="PSUM") as psp,
        ):
            x8_tiles = {}

            def fetch_x8(i):
                if i < NT:
                    t = xtp.tile([P, NCP, 2, P], f8, tag="x8")
                    nc.scalar.dma_start(t[:], d_x8[:, i, :, :, :])
                    x8_tiles[i] = t

            fetch_x8(0)
            c8_tiles = []
            for kt in range(NKT):
                t = cpool.tile([P, NCP, 2, 512], f8, tag=f"c8{kt}", name=f"c8{kt}")
                nc.scalar.dma_start(t[:], d_c8[:, kt, :, :, :])
                c8_tiles.append(t)
                if kt < LOOKAHEAD:
                    fetch_x8(kt + 1)

            for i in range(NT):
                fetch_x8(i + LOOKAHEAD + 1)
                x8_t = x8_tiles.pop(i)
                score_t = scp.tile([P, K], f8, tag="score")
                for q in range(4):
                    ph = psp.tile([P, 1024], dt.float32, tag="ph")
                    for cp in range(NCP):
                        for s in range(2):
                            kt = q * 2 + s
                            nc.tensor.matmul(
                                ph[:, s * 512:(s + 1) * 512],
                                lhsT=x8_t[:, cp, :, :],
                                rhs=c8_tiles[kt][:, cp, :, :],
                                start=(cp == 0), stop=(cp == NCP - 1),
                                perf_mode=mybir.MatmulPerfMode.DoubleRow,
                            )
                    lo, hi = q * 1024, (q + 1) * 1024
                    if q % 2 == 0:
                        nc.scalar.activation(
                            score_t[:, lo:hi], ph[:],
                            mybir.ActivationFunctionType.Identity,
                            scale=SCORE_SCALE,
                        )
                    else:
                        nc.vector.tensor_scalar_mul(
                            score_t[:, lo:hi], ph[:], SCORE_SCALE,
                        )
                    if q == 1:
                        nc.sync.dma_start(
                            d_sc[i * P:(i + 1) * P, 0:2048], score_t[:, 0:2048])
                    elif q == 3:
                        nc.sync.dma_start(
                            d_sc[i * P:(i + 1) * P, 2048:4096], score_t[:, 2048:4096])

    nc.compile()
    return nc


def _build_fp16dr():
    import concourse.bass as bass
    import concourse.mybir as mybir
    from concourse import bacc
    from concourse.tile import TileContext

    dt = mybir.dt
    f32 = dt.float32
    f16 = dt.float16
    f8 = dt.float8e4

    nc = bacc.Bacc("TRN2", target_bir_lowering=False, debug=False)

    d_xm = nc.dram_tensor("xm", [P, NT, NCH, P], f16, kind="ExternalInput").ap()
    d_x8 = nc.dram_tensor("x8", [P, NT, NCH, 2, P], f8, kind="ExternalInput").ap()
    d_cm = nc.dram_tensor("cm", [P, NKT, NCH, 512], f16, kind="ExternalInput").ap()
    d_c8 = nc.dram_tensor("c8", [P, NKT, NCH, 2, 512], f8, kind="ExternalInput").ap()
    d_xn = nc.dram_tensor("xn", [N, D], f32, kind="ExternalInput").ap()
    d_cn = nc.dram_tensor("cn", [K, D], f32, kind="ExternalInput").ap()
    d_out = nc.dram_tensor("out", [N, D], f32, kind="ExternalOutput").ap()

    step1_scale = 2.0 ** -32  # PSUM holds mm * 2^33

    with TileContext(nc) as tc:
        with (
            tc.tile_pool(name="const", bufs=1) as cpool,
            tc.tile_pool(name="xt", bufs=4) as xtp,
            tc.tile_pool(name="xn", bufs=3) as xnp_,
            tc.tile_pool(name="sq", bufs=2) as sqp,
            tc.tile_pool(name="rs", bufs=4) as rsp,
            tc.tile_pool(name="score", bufs=3) as scp,
            tc.tile_pool(name="top", bufs=3) as topp,
            tc.tile_pool(name="gat", bufs=4) as gatp,
            tc.tile_pool(name="psum", bufs=2, space="PSUM") as psp,
        ):
            cm_tiles, c8_tiles = [], []
            for kt in range(NKT):
                tm = cpool.tile([P, NCH, 512], f16, tag=f"cm{kt}", name=f"cm{kt}")
                nc.scalar.dma_start(tm[:], d_cm[:, kt, :, :])
                cm_tiles.append(tm)
                t8 = cpool.tile([P, NCH, 2, 512], f8, tag=f"c8{kt}", name=f"c8{kt}")
                nc.scalar.dma_start(t8[:], d_c8[:, kt, :, :, :])
                c8_tiles.append(t8)

            for i in range(NT):
                xm_t = xtp.tile([P, NCH, P], f16, tag="xm")
                nc.sync.dma_start(xm_t[:], d_xm[:, i, :, :])
                x8_t = xtp.tile([P, NCH, 2, P], f8, tag="x8")
                nc.sync.dma_start(x8_t[:], d_x8[:, i, :, :, :])

                xn_t = xnp_.tile([P, D], f32, tag="xn")
                nc.sync.dma_start(xn_t[:], d_xn[i * P:(i + 1) * P, :])
                sq_t = sqp.tile([P, D], f32, tag="sq")
                rs_t = rsp.tile([P, 1], f32, tag="rs")
                nc.scalar.activation(
                    sq_t[:], xn_t[:], mybir.ActivationFunctionType.Square,
                    accum_out=rs_t[:],
                )
                rsn_t = rsp.tile([P, 1], f32, tag="rsn")
                nc.gpsimd.tensor_scalar_mul(rsn_t[:], rs_t[:], -1.0)

                score_t = scp.tile([P, K], f32, tag="score")
                HKQ = 2048
                for h in range(K // HKQ):
                    ph = psp.tile([P, HKQ], f32, tag="ph")
                    for c in range(NCH):
                        for s in range(HKQ // 512):
                            kt = (h * HKQ) // 512 + s
                            nc.tensor.matmul(
                                ph[:, s * 512:(s + 1) * 512],
                                lhsT=xm_t[:, c, :],
                                rhs=cm_tiles[kt][:, c, :],
                                start=(c == 0), stop=False,
                            )
                    for c in range(NCH):
                        for s in range(HKQ // 512):
                            kt = (h * HKQ) // 512 + s
                            nc.tensor.matmul(
                                ph[:, s * 512:(s + 1) * 512],
                                lhsT=x8_t[:, c, :, :],
                                rhs=c8_tiles[kt][:, c, :, :],
                                start=False, stop=(c == NCH - 1),
                                perf_mode=mybir.MatmulPerfMode.DoubleRow,
                            )
                    nc.scalar.activation(
                        score_t[:, h * HKQ:(h + 1) * HKQ], ph[:],
                        mybir.ActivationFunctionType.Identity,
                        bias=rsn_t[:, 0:1], scale=step1_scale,
                    )

                max8 = topp.tile([P, 8], f32, tag="max8")
                idx8 = topp.tile([P, 8], dt.uint32, tag="idx8")
                nc.vector.max(out=max8[:], in_=score_t[:])
                nc.vector.max_index(out=idx8[:], in_max=max8[:], in_values=score_t[:])

                gat_t = gatp.tile([P, D], f32, tag="gat")
                nc.gpsimd.indirect_dma_start(
                    out=gat_t[:], out_offset=None, in_=d_cn[:],
                    in_offset=bass.IndirectOffsetOnAxis(ap=idx8[:, 0:1], axis=0),
                )
                nc.sync.dma_start(d_out[i * P:(i + 1) * P, :], gat_t[:])

    nc.compile()
    return nc


def _get_nc(mode):
    if mode not in _CACHE:
        builders = {
            "fp8ship": _build_fp8ship,
            "fp8ship_v2": _build_fp8ship_v2,
            "fp8ship_v3": _build_fp8ship_v3,
            "fp8ship_v4": _build_fp8ship_v4,
            "gmax4": _build_gmax4,
            "gmax5": _build_gmax5,
            "gmax6": _build_gmax6,
            "gmax7": _build_gmax7,
            "fp8r2": _build_fp8r2,
            "fp8r3": _build_fp8r3,
            "fp16dr": _build_fp16dr,
        }
        _CACHE[mode] = builders[mode]()
    return _CACHE[mode]


def _prep_xt(x):
    # x: [N, D] -> [P, NT, NCH, P] (partition=d%128, token-tile, d-chunk, token)
    return np.ascontiguousarray(
        x.T.reshape(NCH, P, NT, P).transpose(1, 2, 0, 3)
    )


def _prep_ct(c):
    # c: [K, D] -> [P, NKT, NCH, 512]
    return np.ascontiguousarray(
        c.T.reshape(NCH, P, NKT, 512).transpose(1, 2, 0, 3)
    )


def _prep_x8_pairs(x8):
    # x8: [N, D] fp8 -> [P, NT, NCP, 2, P]; d = (2*cp+q)*128 + p, token = 128*i+t
    a = x8.T.reshape(NCP, 2, P, NT, P)          # [cp, q, p, i, t]
    return np.ascontiguousarray(a.transpose(2, 3, 0, 1, 4))


def _prep_c8_pairs(c8):
    # c8: [K, D] fp8 -> [P, NKT, NCP, 2, 512]
    a = c8.T.reshape(NCP, 2, P, NKT, 512)       # [cp, q, p, kt, kcol]
    return np.ascontiguousarray(a.transpose(2, 3, 0, 1, 4))


def _prep_c8_quads(c8):
    # c8: [K, D] fp8 -> [P, 4, NCP, 2, 1024] (4 super-ktiles of 1024)
    a = c8.T.reshape(NCP, 2, P, 4, 1024)        # [cp, q, p, kt, kcol]
    return np.ascontiguousarray(a.transpose(2, 3, 0, 1, 4))


def _run_spmd(nc, in_maps):
    from concourse.bass_utils import run_bass_kernel_spmd
    try:
        return run_bass_kernel_spmd(nc, in_maps, core_ids=list(range(NCORES)))
    except ModuleNotFoundError:
        # tracing requested but axon ntff hook unavailable in this container
        os.environ["BASS_NEVER_TRACE"] = "1"
        return run_bass_kernel_spmd(nc, in_maps, core_ids=list(range(NCORES)))


def _kernel_fp8ship(inp, codebook, mode="fp8ship"):
    global LAST_RESULT
    import ml_dtypes
    f8np = ml_dtypes.float8_e4m3
    f32 = np.float32

    flat = inp.reshape(-1, D)                      # [32768, 512]
    shards = flat.reshape(NCORES, N, D)

    nc = _get_nc(mode)

    c8 = (codebook * f32(CS)).astype(f8np)
    if mode == "fp8ship_v3":
        c8_p = _prep_c8_quads(c8).view(np.uint8)
    else:
        c8_p = _prep_c8_pairs(c8).view(np.uint8)
    in_maps = []
    for s in range(NCORES):
        x8 = (shards[s] * f32(XS)).astype(f8np)
        x8_p = _prep_x8_pairs(x8).view(np.uint8)
        if mode in ("fp8ship_v2", "fp8ship_v3"):
            x8_p = x8_p.reshape(P, NT // 2, 2, NCP, 2, P)
        elif mode == "fp8ship_v4":
            x8_p = x8_p.reshape(P, NT // 4, 4, NCP, 2, P)
        in_maps.append({"x8": x8_p, "c8": c8_p})

    res = _run_spmd(nc, in_maps)
    LAST_RESULT = res
    if mode == "fp8ship_v4":
        sc_raw = [
            np.asarray(r["sc"]).view(np.uint8)
            .reshape(NT // 2, P, 2, K).transpose(0, 2, 1, 3).reshape(N, K)
            for r in res.results
        ]
    else:
        sc_raw = [np.asarray(r["sc"]).view(np.uint8) for r in res.results]
    scores_u8 = np.concatenate(sc_raw, axis=0)        # [32768, 4096] e4m3 bytes

    # host: candidates = approx scores within MARGIN of each row max
    # (fp8 matmul+quant noise sigma ~4.6e-4 pairwise; margin = 5.4 sigma),
    # then exact rescore in reference arithmetic with first-index tie-break.
    NTOK = flat.shape[0]
    lut = (np.arange(256, dtype=np.uint8).view(ml_dtypes.float8_e4m3)
           .astype(f32) * SCORE_DESCALE)              # e4m3 byte -> score
    s = lut[scores_u8]                                # [32768, 4096] f32
    rowmax = s.max(axis=1)
    rows, cols = np.nonzero(s >= (rowmax[:, None] - MARGIN))

    x64 = flat.astype(np.float64)
    s1 = np.einsum("nd,nd->n", x64, x64).astype(f32)
    mm = np.einsum("id,id->i", flat[rows], codebook[cols])
    d2 = s1[rows] - f32(2.0) * mm
    order = np.lexsort((cols, d2, rows))
    rs = rows[order]
    first = np.searchsorted(rs, np.arange(NTOK))
    win = cols[order][first]

    return codebook[win].reshape(inp.shape).astype(np.float32)


def _kernel_fp16dr(inp, codebook):
    global LAST_RESULT
    import ml_dtypes
    f8np = ml_dtypes.float8_e4m3
    f32 = np.float32

    flat = inp.reshape(-1, D)
    shards = flat.reshape(NCORES, N, D)
    nc = _get_nc("fp16dr")

    cs = codebook * f32(CSCALE)              # c * 2^12
    ch = cs.astype(np.float16)
    cl = (cs - ch.astype(f32)).astype(np.float16)
    cm = (ch.astype(f32) * f32(2.0 ** 10)).astype(np.float16)   # exact
    cl8 = (cl.astype(f32) * f32(2.0 ** 17)).astype(f8np)
    ch8 = (ch.astype(f32) * f32(2.0 ** 6)).astype(f8np)
    cm_p = _prep_ct(cm)
    c8_p = np.stack([_prep_ct(cl8), _prep_ct(ch8)], axis=3).view(np.uint8)
    in_maps = []
    for s in range(NCORES):
        x = shards[s]
        xh = x.astype(np.float16)
        xl = (x - xh.astype(f32)).astype(np.float16)
        xm = (xh.astype(f32) * f32(2.0 ** 11)).astype(np.float16)  # exact
        xh8 = (xh.astype(f32) * f32(2.0 ** 4)).astype(f8np)
        xl8 = (xl.astype(f32) * f32(2.0 ** 15)).astype(f8np)
        x8_p = np.stack([_prep_xt(xh8), _prep_xt(xl8)], axis=3).view(np.uint8)
        in_maps.append({
            "xm": _prep_xt(xm), "x8": x8_p,
            "xn": np.ascontiguousarray(x),
            "cm": cm_p, "c8": c8_p, "cn": codebook,
        })

    res = _run_spmd(nc, in_maps)
    LAST_RESULT = res
    out = np.stack([r["out"] for r in res.results])   # [8, 4096, 512]
    return out.reshape(inp.shape).astype(np.float32)


MARGIN_RAW = np.float32(2700.0)  # raw psum units: 2.5e-3 * 2^20 + fp16 slop


def _kernel_fp8r2(inp, codebook):
    """Host side for fp8r2: raw fp8 for quads 1,3; G4 fp16 maxes for
    quads 0,2."""
    global LAST_RESULT
    import ml_dtypes
    f8np = ml_dtypes.float8_e4m3
    f32 = np.float32

    flat = inp.reshape(-1, D)
    shards = flat.reshape(NCORES, N, D)

    nc = _get_nc("fp8r2")

    c8 = (codebook * f32(CS)).astype(f8np)
    c8_p = _prep_c8_quads(c8).view(np.uint8)
    in_maps = []
    for s in range(NCORES):
        x8 = (shards[s] * f32(XS)).astype(f8np)
        x8_p = _prep_x8_pairs(x8).view(np.uint8).reshape(P, NT // 4, 4, NCP, 2, P)
        in_maps.append({"x8": x8_p, "c8": c8_p})

    res = _run_spmd(nc, in_maps)
    LAST_RESULT = res
    raw_u8 = np.concatenate([
        np.asarray(r["raw"]).view(np.uint8)
        .reshape(NT // 4, P, 4, 2048).transpose(0, 2, 1, 3).reshape(N, 2048)
        for r in res.results], axis=0)
    red = np.concatenate([
        np.asarray(r["red"]).view(np.uint16)
        .reshape(NT // 4, P, 4, 512).transpose(0, 2, 1, 3).reshape(N, 512)
        for r in res.results], axis=0).view(np.float16).astype(f32)

    NTOK = flat.shape[0]
    lut = (np.arange(256, dtype=np.uint8).view(ml_dtypes.float8_e4m3)
           .astype(f32) * SCORE_DESCALE)
    s_raw = lut[raw_u8]                  # quads 1,3: codes 1024-2047|3072-4095
    s_red = red * f32(2.0 ** -20)        # G4 maxes quad0 | quad2

    rowmax = np.maximum(s_raw.max(axis=1), s_red.max(axis=1))
    thr = rowmax - MARGIN

    RAW_CODES = np.concatenate([
        np.arange(1024, 2048), np.arange(3072, 4096)]).astype(np.int64)
    r1, c1 = np.nonzero(s_raw >= thr[:, None])
    rows_a = r1
    cols_a = RAW_CODES[c1]

    r2, g2 = np.nonzero(s_red >= thr[:, None])
    is_q0 = g2 < 256
    rb1 = np.repeat(r2[is_q0], 4)
    cb1 = (4 * g2[is_q0][:, None] + np.arange(4)[None, :]).ravel()
    rb2 = np.repeat(r2[~is_q0], 4)
    cb2 = (2048 + 4 * (g2[~is_q0] - 256)[:, None] + np.arange(4)[None, :]).ravel()

    rows_all = np.concatenate([rows_a, rb1, rb2])
    cols_all = np.concatenate([cols_a, cb1, cb2])

    x64 = flat.astype(np.float64)
    s1 = np.einsum("nd,nd->n", x64, x64).astype(f32)
    mm = np.einsum("id,id->i", flat[rows_all], codebook[cols_all])
    d2 = s1[rows_all] - f32(2.0) * mm
    order = np.lexsort((cols_all, d2, rows_all))
    rs = rows_all[order]
    first = np.searchsorted(rs, np.arange(NTOK))
    win = cols_all[order][first]

    return codebook[win].reshape(inp.shape).astype(np.float32)


def _kernel_gmax6(inp, codebook):
    """Host side for gmax6 (quad codebook layout, remapped segments)."""
    global LAST_RESULT
    import ml_dtypes
    f8np = ml_dtypes.float8_e4m3
    f32 = np.float32

    flat = inp.reshape(-1, D)
    shards = flat.reshape(NCORES, N, D)

    nc = _get_nc("gmax6")

    c8 = (codebook * f32(CS)).astype(f8np)
    c8_p = _prep_c8_quads(c8).view(np.uint8)
    in_maps = []
    for s in range(NCORES):
        x8 = (shards[s] * f32(XS)).astype(f8np)
        x8_p = _prep_x8_pairs(x8).view(np.uint8).reshape(P, NT // 4, 4, NCP, 2, P)
        in_maps.append({"x8": x8_p, "c8": c8_p})

    res = _run_spmd(nc, in_maps)
    LAST_RESULT = res
    raw_u8 = np.concatenate([
        np.asarray(r["raw"]).view(np.uint8)
        .reshape(NT // 4, P, 4, 2560).transpose(0, 2, 1, 3).reshape(N, 2560)
        for r in res.results], axis=0)
    red = np.concatenate([
        np.asarray(r["red"]).view(np.uint16)
        .reshape(NT // 4, P, 4, 512).transpose(0, 2, 1, 3).reshape(N, 512)
        for r in res.results], axis=0).view(np.float16).astype(f32)

    NTOK = flat.shape[0]
    lut = (np.arange(256, dtype=np.uint8).view(ml_dtypes.float8_e4m3)
           .astype(f32) * SCORE_DESCALE)
    s_raw = lut[raw_u8]
    s_red = red * f32(2.0 ** -20)

    rowmax = np.maximum(s_raw.max(axis=1), s_red.max(axis=1))
    thr = rowmax - MARGIN

    # raw cols: [kt0 | quad2 (codes 2048..3071) | kt6 | kt7]
    RAW_CODES = np.concatenate([
        np.arange(0, 512),
        np.arange(2048, 3072),
        np.arange(3072, 3584),
        np.arange(3584, 4096),
    ]).astype(np.int64)
    r1, c1 = np.nonzero(s_raw >= thr[:, None])
    rows_a = r1
    cols_a = RAW_CODES[c1]

    r2, g2 = np.nonzero(s_red >= thr[:, None])
    is_g2 = g2 < 256
    # kt1 pair groups: codes 512 + 2c + {0,1}
    rb1 = np.repeat(r2[is_g2], 2)
    cb1 = (512 + 2 * g2[is_g2][:, None] + np.arange(2)[None, :]).ravel()
    # quad1 G4 groups: codes 1024 + 4c + {0..3}
    rb2 = np.repeat(r2[~is_g2], 4)
    cb2 = (1024 + 4 * (g2[~is_g2] - 256)[:, None] + np.arange(4)[None, :]).ravel()

    rows_all = np.concatenate([rows_a, rb1, rb2])
    cols_all = np.concatenate([cols_a, cb1, cb2])

    x64 = flat.astype(np.float64)
    s1 = np.einsum("nd,nd->n", x64, x64).astype(f32)
    mm = np.einsum("id,id->i", flat[rows_all], codebook[cols_all])
    d2 = s1[rows_all] - f32(2.0) * mm
    order = np.lexsort((cols_all, d2, rows_all))
    rs = rows_all[order]
    first = np.searchsorted(rs, np.arange(NTOK))
    win = cols_all[order][first]

    return codebook[win].reshape(inp.shape).astype(np.float32)


def _kernel_gmax5(inp, codebook):
    """Mixed raw-fp8 / fp16-groupmax device kernel + host rescore.

    Per token the device ships, in descaled (mm * 2^-20) units after
    host decode:
      raw fp8 (SCORE_SCALE'd) for codes kt0, kt2, kt3, kt6, kt7
      fp16 raw-psum group maxes: kt1 in pairs {512+2c, 512+2c+1},
      kt4/kt5 in quads {2048+4c..2048+4c+3}.
    """
    global LAST_RESULT
    import ml_dtypes
    f8np = ml_dtypes.float8_e4m3
    f32 = np.float32

    flat = inp.reshape(-1, D)
    shards = flat.reshape(NCORES, N, D)

    nc = _get_nc("gmax5")

    c8 = (codebook * f32(CS)).astype(f8np)
    c8_p = _prep_c8_pairs(c8).view(np.uint8)
    in_maps = []
    for s in range(NCORES):
        x8 = (shards[s] * f32(XS)).astype(f8np)
        x8_p = _prep_x8_pairs(x8).view(np.uint8).reshape(P, NT // 4, 4, NCP, 2, P)
        in_maps.append({"x8": x8_p, "c8": c8_p})

    res = _run_spmd(nc, in_maps)
    LAST_RESULT = res
    raw_u8 = np.concatenate([
        np.asarray(r["raw"]).view(np.uint8)
        .reshape(NT // 4, P, 4, 2560).transpose(0, 2, 1, 3).reshape(N, 2560)
        for r in res.results], axis=0)              # [32768, 2560]
    red = np.concatenate([
        np.asarray(r["red"]).view(np.uint16)
        .reshape(NT // 4, P, 4, 512).transpose(0, 2, 1, 3).reshape(N, 512)
        for r in res.results], axis=0).view(np.float16).astype(f32)

    NTOK = flat.shape[0]
    lut = (np.arange(256, dtype=np.uint8).view(ml_dtypes.float8_e4m3)
           .astype(f32) * SCORE_DESCALE)            # fp8 byte -> descaled
    s_raw = lut[raw_u8]                             # [N, 2560] descaled
    s_red = red * f32(2.0 ** -20)                   # [N, 512]  descaled

    rowmax = np.maximum(s_raw.max(axis=1), s_red.max(axis=1))
    thr = rowmax - MARGIN

    # raw columns -> codes directly
    RAW_CODES = np.concatenate([
        np.arange(0, 512),          # kt0
        np.arange(1536, 2560),      # kt3, kt4
        np.arange(3072, 3584),      # kt6
        np.arange(3584, 4096),      # kt7
    ]).astype(np.int64)
    r1, c1 = np.nonzero(s_raw >= thr[:, None])
    rows_a = r1
    cols_a = RAW_CODES[c1]

    # reduced columns -> expand groups
    r2, g2 = np.nonzero(s_red >= thr[:, None])
    is_g2 = g2 < 256
    # kt5 pair groups: codes 2560 + 2c + {0,1}
    rb1 = np.repeat(r2[is_g2], 2)
    cb1 = (2560 + 2 * g2[is_g2][:, None] + np.arange(2)[None, :]).ravel()
    # kt1,kt2 quad groups: codes 512 + 4c + {0..3}
    rb2 = np.repeat(r2[~is_g2], 4)
    cb2 = (512 + 4 * (g2[~is_g2] - 256)[:, None] + np.arange(4)[None, :]).ravel()

    rows_all = np.concatenate([rows_a, rb1, rb2])
    cols_all = np.concatenate([cols_a, cb1, cb2])

    x64 = flat.astype(np.float64)
    s1 = np.einsum("nd,nd->n", x64, x64).astype(f32)
    mm = np.einsum("id,id->i", flat[rows_all], codebook[cols_all])
    d2 = s1[rows_all] - f32(2.0) * mm
    order = np.lexsort((cols_all, d2, rows_all))
    rs = rows_all[order]
    first = np.searchsorted(rs, np.arange(NTOK))
    win = cols_all[order][first]

    return codebook[win].reshape(inp.shape).astype(np.float32)


def _kernel_gmax4(inp, codebook):
    global LAST_RESULT
    import ml_dtypes
    f8np = ml_dtypes.float8_e4m3
    f32 = np.float32

    flat = inp.reshape(-1, D)                      # [32768, 512]
    shards = flat.reshape(NCORES, N, D)

    nc = _get_nc("gmax4")

    c8 = (codebook * f32(CS)).astype(f8np)
    c8_p = _prep_c8_pairs(c8).view(np.uint8)
    in_maps = []
    for s in range(NCORES):
        x8 = (shards[s] * f32(XS)).astype(f8np)
        x8_p = _prep_x8_pairs(x8).view(np.uint8).reshape(P, NT // 4, 4, NCP, 2, P)
        in_maps.append({"x8": x8_p, "c8": c8_p})

    res = _run_spmd(nc, in_maps)
    LAST_RESULT = res
    gm_raw = [
        np.asarray(r["gm"]).view(np.uint16)
        .reshape(NT // 2, P, 2, 1024).transpose(0, 2, 1, 3).reshape(N, 1024)
        for r in res.results
    ]
    gm = np.concatenate(gm_raw, axis=0).view(np.float16).astype(f32)
    # gm: [32768, 1024]; column (h*512 + c) = max over codes
    # {2048h + c + 512m, m=0..3} of raw score <x*2^4, c*2^16>.
    NTOK = flat.shape[0]
    rowmax = gm.max(axis=1)
    rows, gcols = np.nonzero(gm >= (rowmax[:, None] - MARGIN_RAW))
    h = gcols >> 9                                  # 0/1
    c = gcols & 511
    # expand each group to its 4 member codes
    rows4 = np.repeat(rows, 4)
    base = (h << 11) + c                            # 2048h + c
    cols4 = (base[:, None] + np.arange(4, dtype=np.int64)[None, :] * 512).ravel()

    x64 = flat.astype(np.float64)
    s1 = np.einsum("nd,nd->n", x64, x64).astype(f32)
    mm = np.einsum("id,id->i", flat[rows4], codebook[cols4])
    d2 = s1[rows4] - f32(2.0) * mm
    order = np.lexsort((cols4, d2, rows4))
    rs = rows4[order]
    first = np.searchsorted(rs, np.arange(NTOK))
    win = cols4[order][first]

    return codebook[win].reshape(inp.shape).astype(np.float32)


def kernel(inp, codebook):
    inp = np.asarray(inp, dtype=np.float32)
    codebook = np.asarray(codebook, dtype=np.float32)
    if MODE == "gmax4":
        return _kernel_gmax4(inp, codebook)
    if MODE == "gmax5":
        return _kernel_gmax5(inp, codebook)
    if MODE == "gmax6":
        return _kernel_gmax6(inp, codebook)
    if MODE == "fp8r2":
        return _kernel_fp8r2(inp, codebook)
    if MODE.startswith("fp8ship"):
        return _kernel_fp8ship(inp, codebook, mode=MODE)
    return _kernel_fp16dr(inp, codebook)

